# revision 40
# baseline (speedup 1.0000x reference)
"""Trainium2 Bass kernel for a 2-layer GNN message-passing encoder.

Math (per layer):  out = segment_mean(x[src] * w, dst) + x @ Wr.T
with w = typew(src,dst) * edge_weight, run twice (Wr1 then Wr2).

Device strategy (8 NeuronCores, SPMD single program), v3 "push-local L2":

  Layer 1 (pull, dst-partitioned edges): core c owns the contiguous
  6272-node dst range. Per 128-node dst window, the weighted segment-mean
  is a chain of one-hot matmuls accumulated in PSUM:
      S[e, n] = (dst_rel[e] == n) * w'[e],   w' = typew * ew / max(cnt,1)
  where S tiles are built ON DEVICE by one fused DVE tensor_scalar per
  tile from a tiny per-slot metadata stream (rel, w'), and the x[src]
  rows are gathered ON HOST into slot order and streamed as fp8 at byte
  rate (no per-edge descriptors). The root linear x @ W1.T is one more
  matmul into the same PSUM chain (lhsT = resident xT).  h goes to DRAM
  (h_slice, fp16) and hT is rebuilt once with a DMA transpose.

  Layer 2 (push-local, src-partitioned edges): each core computes
  partial aggregates for ALL 392 dst windows from the edges whose SRC it
  owns — h[src] rows are gathered from the core's OWN h_slice with the
  SWDGE dma_gather (local rows < 6272, so int16 indices need no class
  split), messages scatter into per-window PSUM via the same masked
  one-hot matmuls, and partials are stored to DRAM in a chunk-major
  window order.  A chunked ReduceScatter (the only collective) then sums
  partials across cores, delivering each core exactly its own 6272 rows.
  The root linear h @ W2.T is precomputed per own-window into a resident
  r2 buffer (PE, from hT) and added to each ReduceScatter chunk on DVE
  before the final store.  Edge tiles are aligned per OCT (8 windows) so
  per-window padding never hits the gather descriptor stream; windows'
  matmul pieces cover the union of tile spans across cores, with
  per-core masking folded into the per-piece S metadata (zero columns
  contribute nothing).

  Queue discipline: SP carries loads + rs reloads, ACT carries stage
  copies / stores / the hT transpose, Pool carries the SWDGE gathers with
  the ReduceScatters emitted after ALL gathers (so a collective's late
  dependencies never head-of-line block gather issue), and the per-chunk
  root adds sit at the end of the DVE stream behind the S builds — each
  queue's program order matches its readiness order.

  partial/rs/out use a partition-major layout ([128, cols]; column block
  w = window w's nodes) so every DMA touching them moves >=512B runs per
  descriptor, dodging the sub-512B half-rate penalty; each chunk's rs
  output is its own tensor because collective outputs must be contiguous.

Host does index/structure work plus one data-movement precompute (the
layer-1 fp8 gather image); all O(E*D) and O(N*D*D) float math runs on
device.
"""

import sys
from contextlib import ExitStack
from dataclasses import dataclass, field

import numpy as np

sys.path.insert(0, "/opt/trn_rl_repo")

import concourse.bacc as bacc  # noqa: E402
import concourse.mybir as mybir  # noqa: E402
import concourse.tile as tile  # noqa: E402
from concourse.bass_utils import run_bass_kernel_spmd  # noqa: E402

D = 128
SAME_W = 0.3
CROSS_W = 1.0


@dataclass
class Cfg:
    n_nodes: int = 50000
    n_cores: int = 8
    ranks_per_core: int = 49
    group: int = 2            # L1 windows per stream batch
    oct: int = 8              # L2 windows per tile-aligned batch
    # L2 ReduceScatter chunk bounds, exclusive prefix of own-rank index
    rs_bounds: tuple = (16, 32, 44)
    # dtype of the exchanged partial aggregates (fp8 halves store+RS bytes;
    # quantization error averages out across the 8-way reduction)
    partial_dtype: str = "float16"
    # fraction of S one-hot builds issued on GPSIMD (Pool) instead of DVE —
    # Pool is idle in L1 and ~60% idle in L2; its per-op cost is ~2.9x DVE's,
    # so a balanced split shortens the build critical path
    pool_build_l1: int = 4   # 1 of every pool_build_l1 builds goes to Pool
    pool_build_l2: int = 8
    # dtype of the host-gathered layer-1 x[src] image
    xg_dtype: str = "float8e4"
    dma_scratch: int = 32768
    gather_tiles_max: int = 32
    n_queues: int = 4
    single_packet: bool = False
    gbufs: int = 4            # L1 xg / L2 gather tile pool depth
    sbufs: int = 3            # S-tile pool depth

    @property
    def npc(self) -> int:
        return self.ranks_per_core * 128

    @property
    def npad(self) -> int:
        return self.n_cores * self.npc

    def rs_chunks(self) -> list:
        """[(r0, r1)] own-rank ranges per ReduceScatter chunk."""
        bounds = sorted({min(b, self.ranks_per_core) for b in self.rs_bounds}
                        | {self.ranks_per_core})
        out = []
        prev = 0
        for b in bounds:
            if b > prev:
                out.append((prev, b))
                prev = b
        return out

    def worder(self) -> np.ndarray:
        """worder[global_window] = processing position (chunk-major:
        chunk k, then owner core, then rank)."""
        RPC = self.ranks_per_core
        chunks = self.rs_chunks()
        pos = np.empty(self.n_cores * RPC, np.int64)
        p = 0
        for (r0, r1) in chunks:
            for c in range(self.n_cores):
                for r in range(r0, r1):
                    pos[c * RPC + r] = p
                    p += 1
        return pos

    def oct_groups(self):
        """L2 batches: consecutive worder positions, never straddling a
        (chunk, core) run. Returns (octs, meta) with meta[i] =
        (chunk_idx, owner_core, rank_start, n_windows)."""
        octs, meta = [], []
        p = 0
        for k, (r0, r1) in enumerate(self.rs_chunks()):
            for c in range(self.n_cores):
                r = r0
                while r < r1:
                    n = min(self.oct, r1 - r)
                    octs.append(list(range(p, p + n)))
                    meta.append((k, c, r, n))
                    p += n
                    r += n
        return octs, meta


@dataclass
class Plan:
    cfg: Cfg
    # layer 1
    T1: np.ndarray        # [RPC] tiles per local dst window
    base1: np.ndarray     # [RPC] first tile of window
    nt1: int = 0
    # layer 2
    T2: np.ndarray = None        # [n_octs] tiles per oct
    base2: np.ndarray = None     # [n_octs] first tile of oct
    nt2: int = 0
    octs: list = field(default_factory=list)     # list of [worder positions]
    pieces: list = field(default_factory=list)   # (wpos, tile_abs, oct_idx)
    ginstrs: list = field(default_factory=list)  # (col, tile_abs, n_tiles, oct)
    idx_cols: int = 0
    npieces: int = 0


def _make_plan(cfg: Cfg, cnt1: np.ndarray, cnt2oct: np.ndarray,
               spans: np.ndarray, octs: list) -> Plan:
    """cnt1: [cores, RPC] L1 per-window counts.
    cnt2oct: [cores, n_octs] L2 per-oct counts.
    spans: [n_windows(worder-ordered), 2] union tile span per window
    (absolute tile indices, computed by caller)."""
    T1 = np.ceil(cnt1.max(axis=0) / 128).astype(np.int64)
    base1 = np.concatenate([[0], np.cumsum(T1)[:-1]])
    nt1 = int(T1.sum())

    n_octs = cnt2oct.shape[1]
    T2 = np.ceil(cnt2oct.max(axis=0) / 128).astype(np.int64)
    base2 = np.concatenate([[0], np.cumsum(T2)[:-1]])
    nt2 = int(T2.sum())

    pieces = []
    for oi, opos in enumerate(octs):
        for wpos in opos:
            lo, hi = spans[wpos]
            for t in range(lo, hi):
                pieces.append((wpos, int(t), oi))

    ginstrs = []
    col = 0
    for oi in range(n_octs):
        t0, n_run = int(base2[oi]), int(T2[oi])
        done = 0
        while done < n_run:
            n = min(cfg.gather_tiles_max, n_run - done)
            ginstrs.append((col, t0 + done, n, oi))
            col += ((n * 8 + 63) // 64) * 64
            done += n
    return Plan(cfg=cfg, T1=T1, base1=base1, nt1=nt1, T2=T2, base2=base2,
                nt2=nt2, octs=octs, pieces=pieces, ginstrs=ginstrs,
                idx_cols=max(col, 64), npieces=len(pieces))


def preprocess(x, edge_index, edge_weight, Wr1, Wr2, cell_len, cfg: Cfg):
    """Host-side index/structure prep. Returns (plan, in_maps)."""
    RPC = cfg.ranks_per_core
    NC = cfg.n_cores
    src = np.asarray(edge_index[0], dtype=np.int64)
    dst = np.asarray(edge_index[1], dtype=np.int64)
    ew = np.asarray(edge_weight, dtype=np.float32)
    cl = int(np.asarray(cell_len))
    x = np.asarray(x, dtype=np.float32)

    tw = np.where((src > cl) == (dst > cl), SAME_W, CROSS_W).astype(np.float32)
    cnt = np.bincount(dst, minlength=cfg.n_nodes).astype(np.float32)
    inv = (1.0 / np.maximum(cnt, 1.0)).astype(np.float32)
    wfin = tw * ew * inv[dst]

    g_dst = dst >> 7                    # global dst window
    core1 = g_dst // RPC                # L1 owner (by dst)
    wl1 = g_dst - core1 * RPC
    cnt1 = np.bincount(core1 * RPC + wl1, minlength=NC * RPC).reshape(NC, RPC)

    # ---- layer 2 structure (by src core, chunk-major window order) ----
    worder = cfg.worder()               # global window -> position
    wpos_e = worder[g_dst]
    core2 = src // cfg.npc              # L2 owner (by src)
    src_local = src - core2 * cfg.npc

    octs, _oct_meta = cfg.oct_groups()
    n_octs = len(octs)
    nw = NC * RPC
    oct_id_of = np.empty(nw, np.int64)       # worder position -> oct index
    oct_first = np.empty(n_octs, np.int64)   # oct -> first worder position
    for oi, opos in enumerate(octs):
        oct_id_of[opos] = oi
        oct_first[oi] = opos[0]
    oct_e = oct_id_of[wpos_e]
    cnt2oct = np.bincount(core2 * n_octs + oct_e,
                          minlength=NC * n_octs).reshape(NC, n_octs)

    # per-core slot position within oct: sort edges by (core2, wpos, src_local)
    order2 = np.lexsort((src_local, wpos_e, core2))
    key2 = (core2 * n_octs + oct_e)[order2]
    starts2 = np.zeros(NC * n_octs + 1, np.int64)
    np.cumsum(np.bincount(key2, minlength=NC * n_octs), out=starts2[1:])
    pos2 = np.arange(len(src)) - starts2[key2]

    # union tile span per window position across cores
    # per (core, wpos): start/end offsets within oct
    wcnt = np.bincount(core2 * nw + wpos_e, minlength=NC * nw).reshape(NC, nw)
    wend = np.cumsum(wcnt, axis=1)      # per core: cumulative end over wpos
    # reset cumsum at oct boundaries: offset within oct
    oct_start_w = oct_first[oct_id_of[np.arange(nw)]]
    base_at_oct = np.where(oct_start_w > 0, wend[:, oct_start_w - 1], 0)
    w_off_end = wend - base_at_oct      # end offset within oct per (core, wpos)
    w_off_start = w_off_end - wcnt

    plan0_T2 = np.ceil(cnt2oct.max(axis=0) / 128).astype(np.int64)
    plan0_base2 = np.concatenate([[0], np.cumsum(plan0_T2)[:-1]])
    has = wcnt > 0
    start_t = np.where(has, w_off_start // 128, np.iinfo(np.int64).max)
    end_t = np.where(has, (w_off_end + 127) // 128, 0)
    lo_w = start_t.min(axis=0)
    hi_w = end_t.max(axis=0)
    spans = np.zeros((nw, 2), np.int64)
    touched = has.any(axis=0)
    oi_w = oct_id_of
    spans[touched, 0] = plan0_base2[oi_w[touched]] + lo_w[touched]
    spans[touched, 1] = plan0_base2[oi_w[touched]] + hi_w[touched]

    plan = _make_plan(cfg, cnt1, cnt2oct, spans, octs)
    nt1, nt2 = plan.nt1, plan.nt2

    # ---- layer 1 slots (per dst core) ----
    order1 = np.lexsort((src, g_dst))
    gid1_s = g_dst[order1]
    starts1 = np.zeros(NC * RPC + 1, np.int64)
    np.cumsum(np.bincount(gid1_s, minlength=NC * RPC), out=starts1[1:])
    pos1 = np.arange(len(src)) - starts1[gid1_s]
    core1_s = core1[order1]
    wl1_s = wl1[order1]
    slot1 = core1_s * (nt1 * 128) + plan.base1[wl1_s] * 128 + pos1

    rel1 = (dst[order1] & 127).astype(np.float32)
    w1v = wfin[order1]
    total1 = NC * nt1 * 128
    rel1_slot = np.full(total1, -1.0, np.float32)
    rel1_slot[slot1] = rel1
    w1_slot = np.zeros(total1, np.float32)
    w1_slot[slot1] = w1v
    src1_slot = np.zeros(total1, np.int64)
    src1_slot[slot1] = src[order1]

    # smeta1: [cores][128, 2*nt1] f32 (col t = rel, col nt1+t = w')
    rel1_pt = rel1_slot.reshape(NC, nt1, 128).transpose(0, 2, 1)
    w1_pt = w1_slot.reshape(NC, nt1, 128).transpose(0, 2, 1)
    smeta1 = np.concatenate([rel1_pt, w1_pt], axis=2)

    # ---- layer 2 slots (per src core) ----
    oct_s = oct_e[order2]
    slot2 = plan.base2[oct_s] * 128 + pos2     # within-core slot
    core2_s = core2[order2]
    total2_core = nt2 * 128
    rel2_slot = np.zeros((NC, total2_core), np.float32)
    w2_slot = np.zeros((NC, total2_core), np.float32)
    win2_slot = np.full((NC, total2_core), -1, np.int64)
    idx2_slot = np.zeros((NC, total2_core), np.int16)
    rel2_slot[core2_s, slot2] = (dst[order2] & 127).astype(np.float32)
    w2_slot[core2_s, slot2] = wfin[order2]
    win2_slot[core2_s, slot2] = wpos_e[order2]
    idx2_slot[core2_s, slot2] = src_local[order2].astype(np.int16)

    # smeta2: per piece p (wpos, t): rel col masked to this window
    piece_w = np.array([p[0] for p in plan.pieces], np.int64)
    piece_t = np.array([p[1] for p in plan.pieces], np.int64)
    npieces = plan.npieces
    rel2_tiles = rel2_slot.reshape(NC, nt2, 128)
    w2_tiles = w2_slot.reshape(NC, nt2, 128)
    win2_tiles = win2_slot.reshape(NC, nt2, 128)
    # [cores, npieces, 128]
    m = win2_tiles[:, piece_t, :] == piece_w[None, :, None]
    rel_p = np.where(m, rel2_tiles[:, piece_t, :], -1.0)
    w_p = np.where(m, w2_tiles[:, piece_t, :], 0.0)
    smeta2 = np.concatenate(
        [rel_p.transpose(0, 2, 1), w_p.transpose(0, 2, 1)], axis=2
    )  # [cores, 128, 2*npieces]

    # device-layout constants
    np_xdt = mybir.dt.np(getattr(mybir.dt, cfg.xg_dtype))
    xnat = np.zeros((cfg.npad, D), np.float32)
    xnat[: cfg.n_nodes] = x
    x8 = xnat.astype(np_xdt)
    iota16 = np.tile(np.arange(128, dtype=np.float16), (128, 1))
    w1t = np.ascontiguousarray(np.asarray(Wr1, np.float16).T)
    w2t = np.ascontiguousarray(np.asarray(Wr2, np.float16).T)

    in_maps = []
    for c in range(NC):
        # layer-1 gathered rows in slot layout [128, nt1*D] (fp8)
        rows = x8[src1_slot[c * nt1 * 128 : (c + 1) * nt1 * 128]]
        xg1 = np.ascontiguousarray(
            rows.reshape(nt1, 128, D).transpose(1, 0, 2).reshape(128, nt1 * D)
        )
        # gather indices for layer 2
        idx_c = idx2_slot[c]
        g16 = np.zeros((16, plan.idx_cols), np.int16)
        for (c0, t0, n_t, _oi) in plan.ginstrs:
            g16[:, c0 : c0 + n_t * 8] = idx_c[t0 * 128 : (t0 + n_t) * 128].reshape(
                -1, 16
            ).T
        gidx = np.ascontiguousarray(np.tile(g16, (8, 1)))
        xT = np.ascontiguousarray(
            xnat[c * cfg.npc : (c + 1) * cfg.npc].astype(np.float16).T
        )
        in_maps.append({
            "xT16": xT,
            "w1t": w1t,
            "w2t": w2t,
            "iota16": iota16,
            "xg1": xg1,
            "gidx": gidx,
            "smeta1": np.ascontiguousarray(smeta1[c]),
            "smeta2": np.ascontiguousarray(smeta2[c]),
        })
    return plan, in_maps


def build_program(plan: Plan, repeat=1):
    cfg = plan.cfg
    RPC = cfg.ranks_per_core
    NC = cfg.n_cores
    dt = mybir.dt
    f32, f16, i16 = dt.float32, dt.float16, dt.int16
    xdt = getattr(dt, cfg.xg_dtype)
    nt1, nt2, npieces = plan.nt1, plan.nt2, plan.npieces

    nc = bacc.Bacc(
        "TRN2",
        target_bir_lowering=False,
        debug=False,
        num_devices=NC,
        dynamic_dma_scratch_size=cfg.dma_scratch,
        num_swdge_queues=cfg.n_queues,
    )
    xT16_d = nc.dram_tensor("xT16", [D, cfg.npc], f16, kind="ExternalInput")
    w1t_d = nc.dram_tensor("w1t", [D, D], f16, kind="ExternalInput")
    w2t_d = nc.dram_tensor("w2t", [D, D], f16, kind="ExternalInput")
    iota_d = nc.dram_tensor("iota16", [128, 128], f16, kind="ExternalInput")
    xg1_d = nc.dram_tensor("xg1", [128, nt1 * D], xdt, kind="ExternalInput")
    gidx_d = nc.dram_tensor("gidx", [128, plan.idx_cols], i16, kind="ExternalInput")
    smeta1_d = nc.dram_tensor("smeta1", [128, 2 * nt1], f32, kind="ExternalInput")
    smeta2_d = nc.dram_tensor("smeta2", [128, 2 * npieces], f32,
                              kind="ExternalInput")
    # out / rs / partial tensors use a partition-major layout — column block
    # w holds window w's 128 nodes, so per-partition DMA runs are >=512B and
    # dodge the sub-512B half-rate descriptor penalty. Each chunk's partial
    # is [NC*128, |R_k|*128]: the ReduceScatter shards the FLAT input, so the
    # leading 128-row blocks are exactly the per-core sections.
    out_d = nc.dram_tensor("out", [128, cfg.npc], f16, kind="ExternalOutput")
    h_slice_d = nc.dram_tensor("h_slice", [cfg.npc, D], f16)
    pdt = getattr(dt, cfg.partial_dtype)
    chunks = cfg.rs_chunks()
    partial_ds = [
        nc.dram_tensor(f"partial{k}", [NC * 128, (r1 - r0) * 128], pdt)
        for k, (r0, r1) in enumerate(chunks)
    ]
    rs_ds = [
        nc.dram_tensor(f"rs{k}", [128, (r1 - r0) * 128], pdt)
        for k, (r0, r1) in enumerate(chunks)
    ]

    Copy = mybir.ActivationFunctionType.Copy
    is_eq, mult = mybir.AluOpType.is_equal, mybir.AluOpType.mult

    octs, oct_meta = cfg.oct_groups()

    grp_pieces = [[] for _ in plan.octs]
    for pi, (wpos, t, oi) in enumerate(plan.pieces):
        grp_pieces[oi].append((pi, wpos, t))
    grp_ginstrs = [[] for _ in plan.octs]
    for inst in plan.ginstrs:
        grp_ginstrs[inst[3]].append(inst)

    max_oct_tiles = int(plan.T2.max())
    max_oct_pieces = max(len(g) for g in grp_pieces)
    groups1 = [list(range(q, min(q + cfg.group, RPC)))
               for q in range(0, RPC, cfg.group)]
    max_grp1_tiles = max(int(sum(plan.T1[w] for w in g)) for g in groups1)

    with tile.TileContext(nc) as tc, ExitStack() as ctx:
        const = ctx.enter_context(tc.tile_pool(name="const", bufs=1))
        g1pool = ctx.enter_context(tc.tile_pool(name="g1", bufs=cfg.gbufs))
        s1pool = ctx.enter_context(tc.tile_pool(name="s1", bufs=cfg.sbufs))
        g2pool = ctx.enter_context(tc.tile_pool(name="g2", bufs=cfg.gbufs))
        s2pool = ctx.enter_context(tc.tile_pool(name="s2", bufs=cfg.sbufs))
        hpool = ctx.enter_context(tc.tile_pool(name="hw", bufs=4))
        rspool = ctx.enter_context(tc.tile_pool(name="rsl", bufs=1))
        psum_w = ctx.enter_context(tc.tile_pool(name="pw", bufs=6, space="PSUM"))
        psum_r = ctx.enter_context(tc.tile_pool(name="pr", bufs=2, space="PSUM"))

        xT_s = const.tile([D, cfg.npc], f16)
        nc.sync.dma_start(xT_s[:], xT16_d[:, :])
        w1t_s = const.tile([D, D], f16)
        nc.sync.dma_start(w1t_s[:], w1t_d[:, :])
        w2t_s = const.tile([D, D], f16)
        nc.sync.dma_start(w2t_s[:], w2t_d[:, :])
        iota_s = const.tile([128, 128], f16)
        nc.sync.dma_start(iota_s[:], iota_d[:, :])
        smeta1_s = const.tile([128, 2 * nt1], f32)
        nc.sync.dma_start(smeta1_s[:], smeta1_d[:, :])
        smeta2_s = const.tile([128, 2 * npieces], f32)
        nc.sync.dma_start(smeta2_s[:], smeta2_d[:, :])
        gidx_s = const.tile([128, plan.idx_cols], i16)
        nc.sync.dma_start(gidx_s[:], gidx_d[:, :])
        hT_s = const.tile([D, cfg.npc], f16)
        r2_s = const.tile([128, RPC, D], f16)

        qn = [0]

        for _rep in range(repeat):
            # ================= layer 1 (pull, local dst windows) ==========
            for grp in groups1:
                grp_t0 = int(plan.base1[grp[0]])
                n_gt = int(sum(plan.T1[w] for w in grp))
                gw = len(grp)
                gt = g1pool.tile([128, max_grp1_tiles, D], xdt, tag="g1")
                sg = s1pool.tile([128, max_grp1_tiles, D], f16, tag="s1")
                if n_gt > 0:
                    nc.sync.dma_start(
                        gt[:, 0:n_gt, :],
                        xg1_d[:, grp_t0 * D : (grp_t0 + n_gt) * D],
                    )
                    for o in range(n_gt):
                        t_abs = grp_t0 + o
                        eng = (nc.gpsimd if cfg.pool_build_l1 and
                               t_abs % cfg.pool_build_l1 == 0 else nc.vector)
                        eng.tensor_scalar(
                            sg[:, o, :],
                            iota_s[:],
                            smeta1_s[:, t_abs : t_abs + 1],
                            smeta1_s[:, nt1 + t_abs : nt1 + t_abs + 1],
                            is_eq,
                            mult,
                        )
                stage = hpool.tile([128, cfg.group, D], f16, tag="hstage")
                pw = psum_w.tile([128, cfg.group, D], f32, tag="pw")
                for wi, wl in enumerate(grp):
                    nc.tensor.matmul(
                        pw[:, wi, :],
                        xT_s[:, wl * 128 : (wl + 1) * 128],
                        w1t_s[:],
                        start=True,
                        stop=(plan.T1[wl] == 0),
                    )
                    for j in range(int(plan.T1[wl])):
                        o = int(plan.base1[wl]) - grp_t0 + j
                        nc.tensor.matmul(
                            pw[:, wi, :],
                            sg[:, o, :],
                            gt[:, o, :],
                            start=False,
                            stop=(j == int(plan.T1[wl]) - 1),
                        )
                nc.scalar.activation(stage[:, 0:gw, :], pw[:, 0:gw, :], Copy)
                r0, r1 = grp[0] * 128, (grp[-1] + 1) * 128
                nc.scalar.dma_start(
                    h_slice_d[r0:r1, :].rearrange("(w p) d -> p w d", p=128),
                    stage[:, 0:gw, :],
                )

            # hT rebuild (one DMA transpose, after all h stores)
            nc.scalar.dma_start_transpose(hT_s[:], h_slice_d[:, :])

            # r2 = h @ W2.T per own window (PE from hT; overlaps layer 2)
            for r0b in range(0, RPC, 4):
                nb = min(4, RPC - r0b)
                pr = psum_r.tile([128, 4, D], f32, tag="pr")
                for ri in range(nb):
                    r = r0b + ri
                    nc.tensor.matmul(
                        pr[:, ri, :], hT_s[:, r * 128 : (r + 1) * 128], w2t_s[:],
                        start=True, stop=True,
                    )
                nc.scalar.activation(r2_s[:, r0b : r0b + nb, :], pr[:, 0:nb, :],
                                     Copy)

            # ================= layer 2 (push-local, all 392 windows) ======
            for oi, opos in enumerate(plan.octs):
                n_gt = int(plan.T2[oi])
                t0 = int(plan.base2[oi])
                gt = g2pool.tile([128, max_oct_tiles, D], f16, tag="g2")
                sg = s2pool.tile([128, max_oct_pieces, D], f16, tag="s2")
                if n_gt > 0:
                    for (c0, gt0, n_t, _oi) in grp_ginstrs[oi]:
                        nc.gpsimd.dma_gather(
                            gt[:, gt0 - t0 : gt0 - t0 + n_t, :],
                            h_slice_d[:, :],
                            gidx_s[:, c0 : c0 + n_t * 8],
                            n_t * 128,
                            n_t * 128,
                            D,
                            queue_num=qn[0],
                            single_packet=cfg.single_packet,
                        )
                        qn[0] = (qn[0] + 1) % cfg.n_queues
                for k, (pi, wpos, t) in enumerate(grp_pieces[oi]):
                    eng = (nc.gpsimd if cfg.pool_build_l2 and
                           pi % cfg.pool_build_l2 == 0 else nc.vector)
                    eng.tensor_scalar(
                        sg[:, k, :],
                        iota_s[:],
                        smeta2_s[:, pi : pi + 1],
                        smeta2_s[:, npieces + pi : npieces + pi + 1],
                        is_eq,
                        mult,
                    )
                pstage = hpool.tile([128, cfg.oct, D], pdt, tag="pstage")
                for w0 in range(0, len(opos), 4):
                    sub = opos[w0 : w0 + 4]
                    nb = len(sub)
                    wps = [[k for k, (pi, w, t) in enumerate(grp_pieces[oi])
                            if w == wpos] for wpos in sub]
                    all_full = all(wps)
                    pw = psum_w.tile([128, 4, D], f32, tag="pw")
                    for wi, (wpos, wp) in enumerate(zip(sub, wps)):
                        if not wp:
                            nc.vector.memset(pstage[:, w0 + wi, :], 0.0)
                            continue
                        for j, k in enumerate(wp):
                            _pi, _w, t = grp_pieces[oi][k]
                            nc.tensor.matmul(
                                pw[:, wi, :],
                                sg[:, k, :],
                                gt[:, t - t0, :],
                                start=(j == 0),
                                stop=(j == len(wp) - 1),
                            )
                        if not all_full:
                            nc.scalar.activation(pstage[:, w0 + wi, :],
                                                 pw[:, wi, :], Copy)
                    if all_full:
                        nc.scalar.activation(pstage[:, w0 : w0 + nb, :],
                                             pw[:, 0:nb, :], Copy)
                ck, cw, rst, nwo = oct_meta[oi]
                c0 = (rst - chunks[ck][0]) * 128
                nc.scalar.dma_start(
                    partial_ds[ck][cw * 128 : (cw + 1) * 128,
                                   c0 : c0 + nwo * 128].rearrange(
                        "p (w d) -> p w d", d=D),
                    pstage[:, 0:nwo, :],
                )

            # ============== ReduceScatter chunks + root add ===============
            for k, (r0, r1) in enumerate(chunks):
                nc.gpsimd.collective_compute(
                    "ReduceScatter",
                    mybir.AluOpType.add,
                    replica_groups=[list(range(NC))],
                    ins=[partial_ds[k][:, :]],
                    outs=[rs_ds[k][:, :].rearrange("p (w d) -> p w d", d=D)],
                )
            max_nw = max(r1 - r0 for (r0, r1) in chunks)
            for k, (r0, r1) in enumerate(chunks):
                nw_k = r1 - r0
                rsld = rspool.tile([128, max_nw, D], pdt, tag="rsld")
                nc.sync.dma_start(
                    rsld[:, 0:nw_k, :],
                    rs_ds[k][:, :].rearrange("p (w d) -> p w d", d=D),
                )
                ost = rspool.tile([128, max_nw, D], f16, tag="ost")
                nc.vector.tensor_add(ost[:, 0:nw_k, :], rsld[:, 0:nw_k, :],
                                     r2_s[:, r0:r1, :])
                nc.scalar.dma_start(
                    out_d[:, r0 * 128 : r1 * 128].rearrange(
                        "p (w d) -> p w d", d=D),
                    ost[:, 0:nw_k, :],
                )

    nc.compile()
    return nc


_CACHE: dict = {}


def _get_program(plan: Plan):
    key = (
        plan.cfg.n_nodes,
        plan.cfg.n_cores,
        plan.cfg.ranks_per_core,
        tuple(plan.T1.tolist()),
        tuple(plan.T2.tolist()),
        plan.npieces,
    )
    if key not in _CACHE:
        _CACHE[key] = build_program(plan)
    return _CACHE[key]


def kernel(x, edge_index, edge_weight, Wr1, Wr2, cell_len):
    cfg = Cfg()
    assert x.shape == (cfg.n_nodes, D)
    plan, in_maps = preprocess(x, edge_index, edge_weight, Wr1, Wr2, cell_len, cfg)
    nc = _get_program(plan)
    res = run_bass_kernel_spmd(nc, in_maps, list(range(cfg.n_cores)))
    # out is partition-major [128, npc]: node w*128+p lives at [p, w*128:...]
    out = np.concatenate(
        [
            res.results[c]["out"]
            .reshape(128, cfg.ranks_per_core, D)
            .transpose(1, 0, 2)
            .reshape(cfg.npc, D)
            for c in range(cfg.n_cores)
        ],
        axis=0,
    )
    return np.ascontiguousarray(out[: cfg.n_nodes]).astype(np.float32)
